# revision 1
# baseline (speedup 1.0000x reference)
"""Galerkin-attention encoder block on 8 TRN2 NeuronCores.

Sharding: tokens (N=8192 -> 1024/core). The only cross-core dependency is
the Galerkin contraction scores[b,h] = sum_n k[n] (x) v[n] / N, reduced with
four per-batch 512KB AllReduces that overlap local compute.

All device compute runs in "transposed space" (features on partitions,
tokens on the free axis) against a host-side pre-transposed bf16 x^T, so
the kernel needs no on-device transposes anywhere:
  qT = Wq^T x^T, attnT = scores^T qT, x1T = xT + attnT,
  hT = silu(W1^T x1T), outT = x1T + W2^T hT.
k and v stay in [token, feature] layout (their LayerNorm reduces along the
free axis and the scores matmul contracts over tokens = partitions).
"""

import numpy as np
import ml_dtypes

B, N, D = 4, 8192, 1024
H, DK = 8, 128
FF = 4096
EPS = 1e-5
N_CORES = 8
NT = N // N_CORES          # tokens per core
KC = D // 128              # feature chunks of 128
FC = FF // 128
SUP = 512                  # tokens per super-tile in phases B1/B2
NSUP = NT // SUP
SUB = 128                  # tokens per sub-tile in phase A
NSUB = SUP // SUB

_GRAPH_CACHE = {}


def _build(flags, phases=3):
    import concourse.bass as bass
    import concourse.tile as tile
    from concourse import bacc, mybir
    from contextlib import ExitStack

    has_bk, has_bv, has_b2, has_affine = flags
    f32 = mybir.dt.float32
    bf16 = mybir.dt.bfloat16

    nc = bacc.Bacc("TRN2", target_bir_lowering=False, debug=False,
                   num_devices=N_CORES)

    xTb_d = nc.dram_tensor("xTb", [B, D, NT], bf16, kind="ExternalInput")
    delta_d = nc.dram_tensor("delta", [NT], f32, kind="ExternalInput")
    wq_d = nc.dram_tensor("Wq", [D, D], bf16, kind="ExternalInput")
    wk_d = nc.dram_tensor("Wk", [D, D], bf16, kind="ExternalInput")
    wv_d = nc.dram_tensor("Wv", [D, D], bf16, kind="ExternalInput")
    w1_d = nc.dram_tensor("W1", [D, FF], bf16, kind="ExternalInput")
    w2_d = nc.dram_tensor("W2", [FF, D], bf16, kind="ExternalInput")
    bq_d = nc.dram_tensor("bq", [D], f32, kind="ExternalInput")
    b1_d = nc.dram_tensor("b1", [FF], f32, kind="ExternalInput")
    bk_d = nc.dram_tensor("bk", [D], f32, kind="ExternalInput") if has_bk else None
    bv_d = nc.dram_tensor("bv", [D], f32, kind="ExternalInput") if has_bv else None
    b2_d = nc.dram_tensor("b2", [D], f32, kind="ExternalInput") if has_b2 else None
    gamma_d = nc.dram_tensor("gamma", [D], f32, kind="ExternalInput") if has_affine else None
    beta_d = nc.dram_tensor("beta", [D], f32, kind="ExternalInput") if has_affine else None
    out_d = nc.dram_tensor("outT", [B, D, NT], f32, kind="ExternalOutput")

    sub_ = mybir.AluOpType.subtract
    mult = mybir.AluOpType.mult
    ACT = mybir.ActivationFunctionType

    with tile.TileContext(nc) as tc, ExitStack() as ctx:
        singles = ctx.enter_context(tc.tile_pool(name="singles", bufs=1))
        dram = ctx.enter_context(tc.tile_pool(name="dram", bufs=1, space="DRAM"))

        eps_t = singles.tile([128, 1], f32)
        nc.vector.memset(eps_t, EPS)
        delta_sb = singles.tile([128, NT // 128], f32)
        nc.sync.dma_start(out=delta_sb[:], in_=delta_d.ap().rearrange("(g p) -> p g", p=128))
        scores_bf = singles.tile([128, B, H, DK], bf16)

        cc_in = dram.tile([B, 128, H * DK], f32)
        cc_out = [dram.tile([128, H * DK], f32, addr_space="Shared",
                            name=f"cc_out{b}") for b in range(B)]
        h_dram = dram.tile([B, NSUP, 128, FC, SUP], bf16)
        x1_dram = dram.tile([B, NSUP, 128, KC, SUP], bf16)

        # First half of W2, reserved below w_ab1 on the pool stack so it
        # survives into B2; its DMA rides the scalar queue early, letting
        # B2's matmuls start the moment B1's weights release.
        w_b2a_cm = tc.tile_pool(name="w_b2a", bufs=1)
        w_b2a = w_b2a_cm.__enter__()
        w2a_sb = w_b2a.tile([128, FC // 2, D], bf16)
        nc.scalar.dma_start(
            out=w2a_sb[:],
            in_=w2_d.ap()[0:FF // 2].rearrange("(kc p) f -> p kc f", p=128))

        # Weights that live through phases A+B1. Loaded on the Scalar
        # engine's DMA queue so they don't delay Wk/Wv/x on sync — those
        # gate the very first matmuls.
        w_ab1_cm = tc.tile_pool(name="w_ab1", bufs=1)
        w_ab1 = w_ab1_cm.__enter__()
        wq_sb = w_ab1.tile([128, KC, D], bf16)
        nc.scalar.dma_start(out=wq_sb[:], in_=wq_d.ap().rearrange("(kc p) f -> p kc f", p=128))
        w1_sb = w_ab1.tile([128, KC, FF], bf16)
        nc.scalar.dma_start(out=w1_sb[:], in_=w1_d.ap().rearrange("(kc p) f -> p kc f", p=128))
        bq_sb = w_ab1.tile([128, KC], f32)
        nc.scalar.dma_start(out=bq_sb[:], in_=bq_d.ap().rearrange("(m p) -> p m", p=128))
        b1_sb = w_ab1.tile([128, FC], f32)
        nc.scalar.dma_start(out=b1_sb[:], in_=b1_d.ap().rearrange("(m p) -> p m", p=128))
        if has_affine:
            gamma_sb = w_ab1.tile([128, D], f32)
            nc.scalar.dma_start(out=gamma_sb[:], in_=gamma_d.ap().to_broadcast([128, D]))
            beta_sb = w_ab1.tile([128, D], f32)
            nc.scalar.dma_start(out=beta_sb[:], in_=beta_d.ap().to_broadcast([128, D]))
        if has_bk:
            bk_sb = w_ab1.tile([128, D], f32)
            nc.scalar.dma_start(out=bk_sb[:], in_=bk_d.ap().to_broadcast([128, D]))
        if has_bv:
            bv_sb = w_ab1.tile([128, D], f32)
            nc.scalar.dma_start(out=bv_sb[:], in_=bv_d.ap().to_broadcast([128, D]))

        xTb_r = [xTb_d.ap()[b].rearrange("(kc p) t -> p kc t", p=128) for b in range(B)]

        # ---------------- Phase A: k, v, LN, partial scores, AllReduce ----
        with (
            tc.tile_pool(name="wa", bufs=1) as wa,
            tc.tile_pool(name="a_x", bufs=2) as a_x,
            tc.tile_pool(name="a_kvf", bufs=4) as a_kvf,
            tc.tile_pool(name="a_ln", bufs=4) as a_ln,
            tc.tile_pool(name="a_sc", bufs=2) as a_sc,
            tc.tile_pool(name="a_kvps", bufs=3, space="PSUM") as a_kvps,
            tc.tile_pool(name="a_sps", bufs=1, space="PSUM") as a_sps,
        ):
            wk_sb = wa.tile([128, KC, D], bf16)
            nc.sync.dma_start(out=wk_sb[:], in_=wk_d.ap().rearrange("(kc p) f -> p kc f", p=128))
            wv_sb = wa.tile([128, KC, D], bf16)
            nc.sync.dma_start(out=wv_sb[:], in_=wv_d.ap().rearrange("(kc p) f -> p kc f", p=128))

            for b in range(B):
                scores_ps = a_sps.tile([128, H, DK], f32, tag="scores")
                # Four per-head accumulation groups share each PSUM bank, and
                # a start=True matmul clears its whole bank — zero once and
                # accumulate with start=False instead.
                nc.vector.memset(scores_ps, 0.0)
                for s in range(NSUP):
                    xtb = a_x.tile([128, KC, SUP], bf16, tag="xtb")
                    nc.sync.dma_start(out=xtb[:], in_=xTb_r[b][:, :, s * SUP:(s + 1) * SUP])
                    for sb in range(NSUB):
                        gsub = s * NSUB + sb
                        tsl = bass.ts(sb, SUB)

                        def proj(w_sb, bias_sb, tag):
                            ps = a_kvps.tile([128, D], f32, tag="kv")
                            for oc in range(2):
                                for kc in range(KC):
                                    nc.tensor.matmul(
                                        ps[:, oc * 512:(oc + 1) * 512],
                                        lhsT=xtb[:, kc, tsl],
                                        rhs=w_sb[:, kc, oc * 512:(oc + 1) * 512],
                                        start=(kc == 0), stop=(kc == KC - 1))
                            if bias_sb is not None:
                                nc.vector.tensor_add(ps[:], ps[:], bias_sb[:])
                            return ps

                        k_ps = proj(wk_sb, bk_sb if has_bk else None, "k")
                        v_ps = proj(wv_sb, bv_sb if has_bv else None, "v")

                        def layernorm(ps, with_delta, tag):
                            # DVE computes the stats; the LN apply runs on the
                            # Scalar engine as Identity(k*rstd + (-mu*rstd)) so
                            # DVE doesn't gate the PE in phase A.
                            stats = a_ln.tile([128, H, 6], f32, tag="stats", name=f"stats{tag}")
                            mv = a_ln.tile([128, H, 2], f32, tag="mv", name=f"mv{tag}")
                            for h in range(H):
                                nc.vector.bn_stats(out=stats[:, h, :], in_=ps[:, h * DK:(h + 1) * DK])
                                nc.vector.bn_aggr(out=mv[:, h, :], in_=stats[:, h, :])
                            rstd = a_ln.tile([128, H], f32, tag="rstd", name=f"rstd{tag}")
                            nc.scalar.activation(out=rstd[:], in_=mv[:, :, 1], func=ACT.Sqrt, bias=eps_t[:])
                            nc.vector.reciprocal(out=rstd[:], in_=rstd[:])
                            out_t = a_kvf.tile([128, D], bf16, tag="kvf", name=f"kvf{tag}")
                            if not has_affine:
                                if with_delta:
                                    nc.vector.tensor_scalar_mul(
                                        out=rstd[:], in0=rstd[:],
                                        scalar1=delta_sb[:, gsub:gsub + 1])
                                nmr = a_ln.tile([128, H], f32, tag="nmr", name=f"nmr{tag}")
                                nc.vector.tensor_mul(nmr[:], mv[:, :, 0], rstd[:])
                                nc.vector.tensor_scalar_mul(out=nmr[:], in0=nmr[:], scalar1=-1.0)
                                for h in range(H):
                                    hs = slice(h * DK, (h + 1) * DK)
                                    nc.scalar.activation(
                                        out=out_t[:, hs], in_=ps[:, hs], func=ACT.Identity,
                                        bias=nmr[:, h:h + 1], scale=rstd[:, h:h + 1])
                            else:
                                for h in range(H):
                                    hs = slice(h * DK, (h + 1) * DK)
                                    nc.vector.tensor_scalar(
                                        out=out_t[:, hs], in0=ps[:, hs],
                                        scalar1=mv[:, h, 0:1], scalar2=rstd[:, h:h + 1],
                                        op0=sub_, op1=mult)
                                nc.vector.tensor_mul(out_t[:], out_t[:], gamma_sb[:])
                                nc.vector.tensor_add(out_t[:], out_t[:], beta_sb[:])
                                if with_delta:
                                    nc.vector.tensor_scalar_mul(
                                        out=out_t[:], in0=out_t[:],
                                        scalar1=delta_sb[:, gsub:gsub + 1])
                            return out_t

                        kf = layernorm(k_ps, True, "k")
                        vf = layernorm(v_ps, False, "v")

                        for h in range(H):
                            hs = slice(h * DK, (h + 1) * DK)
                            nc.tensor.matmul(
                                scores_ps[:, h, :], lhsT=kf[:, hs], rhs=vf[:, hs],
                                start=False,
                                stop=(s == NSUP - 1 and sb == NSUB - 1),
                                skip_group_check=True)

                sc_sb = a_sc.tile([128, H * DK], f32, tag="scsb")
                nc.vector.tensor_copy(out=sc_sb[:], in_=scores_ps[:, :, :])
                nc.sync.dma_start(out=cc_in[b], in_=sc_sb[:])
                nc.gpsimd.collective_compute(
                    "AllReduce", mybir.AluOpType.add,
                    replica_groups=[list(range(N_CORES))],
                    ins=[cc_in[b].opt()], outs=[cc_out[b].opt()])

        if phases < 1:
            # Debug: dump reduced scores straight to outT and stop.
            with tc.tile_pool(name="dbg", bufs=1) as dbg:
                sc_dbg = dbg.tile([128, B, H * DK], f32)
                for b in range(B):
                    nc.sync.dma_start(out=sc_dbg[:, b, :], in_=cc_out[b])
                    nc.sync.dma_start(out=out_d.ap()[b][0:128, :],
                                      in_=sc_dbg[:, b, 0:NT])
            w_ab1_cm.__exit__(None, None, None)
            nc.finalize()
            return nc

        # ---------------- Phase B1: qT, attn, x1T, first FFN matmul -------
        with (
            tc.tile_pool(name="b1_x", bufs=2) as b1_x,
            tc.tile_pool(name="b1_q", bufs=1) as b1_q,
            tc.tile_pool(name="b1_x1", bufs=2) as b1_x1,
            tc.tile_pool(name="b1_h", bufs=2) as b1_h,
            tc.tile_pool(name="b1_qps", bufs=2, space="PSUM") as b1_qps,
            tc.tile_pool(name="b1_aps", bufs=2, space="PSUM") as b1_aps,
            tc.tile_pool(name="b1_hps", bufs=3, space="PSUM") as b1_hps,
        ):
            with tc.tile_pool(name="b1_sc", bufs=2) as b1_sc:
                for b in range(B):
                    sc_f = b1_sc.tile([128, H * DK], f32, tag="scf")
                    nc.sync.dma_start(out=sc_f[:], in_=cc_out[b])
                    nc.vector.tensor_copy(out=scores_bf[:, b, :, :], in_=sc_f[:])

            for b in range(B):
                for s in range(NSUP):
                    xtb = b1_x.tile([128, KC, SUP], bf16, tag="xtb1")
                    nc.sync.dma_start(out=xtb[:], in_=xTb_r[b][:, :, s * SUP:(s + 1) * SUP])

                    qt = b1_q.tile([128, H, SUP], bf16, tag="qt")
                    for m in range(KC):
                        q_ps = b1_qps.tile([128, SUP], f32, tag="qps")
                        for kc in range(KC):
                            nc.tensor.matmul(
                                q_ps[:], lhsT=wq_sb[:, kc, m * 128:(m + 1) * 128],
                                rhs=xtb[:, kc, :],
                                start=(kc == 0), stop=(kc == KC - 1))
                        nc.scalar.activation(out=qt[:, m, :], in_=q_ps[:],
                                             func=ACT.Identity, bias=bq_sb[:, m:m + 1])

                    x1 = b1_x1.tile([128, KC, SUP], bf16, tag="x1")
                    for h in range(H):
                        a_ps = b1_aps.tile([128, SUP], f32, tag="aps")
                        nc.tensor.matmul(a_ps[:], lhsT=scores_bf[:, b, h, :],
                                         rhs=qt[:, h, :], start=True, stop=True)
                        nc.vector.tensor_add(x1[:, h, :], a_ps[:], xtb[:, h, :])
                    nc.gpsimd.dma_start(out=x1_dram[b, s], in_=x1[:])

                    for g in range(4):
                        hsb = b1_h.tile([128, 8, SUP], bf16, tag="hsb")
                        for mm in range(8):
                            m = g * 8 + mm
                            h_ps = b1_hps.tile([128, SUP], f32, tag="hps")
                            for kc in range(KC):
                                nc.tensor.matmul(
                                    h_ps[:], lhsT=w1_sb[:, kc, m * 128:(m + 1) * 128],
                                    rhs=x1[:, kc, :],
                                    start=(kc == 0), stop=(kc == KC - 1))
                            nc.scalar.activation(out=hsb[:, mm, :], in_=h_ps[:],
                                                 func=ACT.Silu, bias=b1_sb[:, m:m + 1])
                        nc.gpsimd.dma_start(out=h_dram[b, s, :, g * 8:(g + 1) * 8, :], in_=hsb[:])

        w_ab1_cm.__exit__(None, None, None)

        if phases < 2:
            w_b2a_cm.__exit__(None, None, None)
            # Debug: dump x1 straight to outT and stop.
            with tc.tile_pool(name="dbg2", bufs=2) as dbg2:
                for b in range(B):
                    for s in range(NSUP):
                        x1d = dbg2.tile([128, KC, SUP], bf16, tag="x1d")
                        nc.sync.dma_start(out=x1d[:], in_=x1_dram[b, s])
                        x1f = dbg2.tile([128, KC, SUP], f32, tag="x1f")
                        nc.vector.tensor_copy(out=x1f[:], in_=x1d[:])
                        out_r_b = out_d.ap()[b].rearrange("(m p) t -> p m t", p=128)
                        nc.sync.dma_start(out=out_r_b[:, :, s * SUP:(s + 1) * SUP],
                                          in_=x1f[:])
            nc.finalize()
            return nc

        # ---------------- Phase B2: yT + residual -------------------------
        with (
            tc.tile_pool(name="w_b2", bufs=1) as w_b2,
            tc.tile_pool(name="b2_h", bufs=2) as b2_h,
            tc.tile_pool(name="b2_x1", bufs=2) as b2_x1,
            tc.tile_pool(name="b2_o", bufs=2) as b2_o,
            tc.tile_pool(name="b2_yps", bufs=4, space="PSUM") as b2_yps,
        ):
            w2b_sb = w_b2.tile([128, FC // 2, D], bf16)
            nc.sync.dma_start(
                out=w2b_sb[:],
                in_=w2_d.ap()[FF // 2:].rearrange("(kc p) f -> p kc f", p=128))
            if has_b2:
                b2_sb = w_b2.tile([128, KC], f32)
                nc.sync.dma_start(out=b2_sb[:], in_=b2_d.ap().rearrange("(m p) -> p m", p=128))

            out_r = [out_d.ap()[b].rearrange("(m p) t -> p m t", p=128) for b in range(B)]
            FCH = FC // 2
            for b in range(B):
                for s in range(NSUP):
                    ht = b2_h.tile([128, FC, SUP], bf16, tag="ht")
                    nc.sync.dma_start(out=ht[:], in_=h_dram[b, s])
                    x1r = b2_x1.tile([128, KC, SUP], bf16, tag="x1r")
                    nc.sync.dma_start(out=x1r[:], in_=x1_dram[b, s])
                    ot = b2_o.tile([128, KC, SUP], f32, tag="ot")
                    for m in range(KC):
                        y_ps = b2_yps.tile([128, SUP], f32, tag="yps")
                        for kc in range(FCH):
                            nc.tensor.matmul(
                                y_ps[:], lhsT=w2a_sb[:, kc, m * 128:(m + 1) * 128],
                                rhs=ht[:, kc, :],
                                start=(kc == 0), stop=False)
                        for kc in range(FCH):
                            nc.tensor.matmul(
                                y_ps[:], lhsT=w2b_sb[:, kc, m * 128:(m + 1) * 128],
                                rhs=ht[:, FCH + kc, :],
                                start=False, stop=(kc == FCH - 1))
                        if has_b2:
                            nc.vector.tensor_scalar_add(out=y_ps[:], in0=y_ps[:],
                                                        scalar1=b2_sb[:, m:m + 1])
                        nc.vector.tensor_add(ot[:, m, :], y_ps[:], x1r[:, m, :])
                    nc.gpsimd.dma_start(out=out_r[b][:, :, s * SUP:(s + 1) * SUP], in_=ot[:])

        w_b2a_cm.__exit__(None, None, None)

    nc.finalize()
    return nc


def _get_graph(flags):
    if flags not in _GRAPH_CACHE:
        _GRAPH_CACHE[flags] = _build(flags)
    return _GRAPH_CACHE[flags]


def kernel(x, delta_x, Wq, bq, Wk, bk, Wv, bv, gamma_k, beta_k, W1, b1, W2, b2,
           _trace=False):
    from concourse.bass_utils import run_bass_kernel_spmd

    bf = ml_dtypes.bfloat16
    x = np.asarray(x, np.float32)
    delta_x = np.asarray(delta_x, np.float32)
    Wq, Wk, Wv = (np.asarray(w, np.float32) for w in (Wq, Wk, Wv))
    W1, W2 = np.asarray(W1, np.float32), np.asarray(W2, np.float32)
    bq, bk, bv = (np.asarray(v, np.float32) for v in (bq, bk, bv))
    b1, b2 = np.asarray(b1, np.float32), np.asarray(b2, np.float32)
    gamma_k = np.asarray(gamma_k, np.float32)
    beta_k = np.asarray(beta_k, np.float32)

    has_bk = bool(np.any(bk))
    has_bv = bool(np.any(bv))
    has_b2 = bool(np.any(b2))
    has_affine = not (np.all(gamma_k == 1.0) and np.all(beta_k == 0.0))
    flags = (has_bk, has_bv, has_b2, has_affine)
    nc = _get_graph(flags)

    wq_b, wk_b, wv_b = Wq.astype(bf), Wk.astype(bf), Wv.astype(bf)
    w1_b, w2_b = W1.astype(bf), W2.astype(bf)
    delta_pre = (delta_x / np.float32(N)).astype(np.float32)

    in_maps = []
    for c in range(N_CORES):
        t0 = c * NT
        xT = np.ascontiguousarray(x[:, t0:t0 + NT, :].transpose(0, 2, 1)).astype(bf)
        m = {"xTb": xT, "delta": np.ascontiguousarray(delta_pre[t0:t0 + NT]),
             "Wq": wq_b, "Wk": wk_b, "Wv": wv_b, "W1": w1_b, "W2": w2_b,
             "bq": bq, "b1": b1}
        if has_bk:
            m["bk"] = bk
        if has_bv:
            m["bv"] = bv
        if has_b2:
            m["b2"] = b2
        if has_affine:
            m["gamma"] = gamma_k.reshape(D).copy()
            m["beta"] = beta_k.reshape(D).copy()
        in_maps.append(m)

    res = run_bass_kernel_spmd(nc, in_maps, core_ids=list(range(N_CORES)),
                               trace=_trace)

    out = np.empty((B, N, D), np.float32)
    for c in range(N_CORES):
        t0 = c * NT
        out[:, t0:t0 + NT, :] = res.results[c]["outT"].transpose(0, 2, 1)
    if _trace:
        return out, res
    return out



# revision 2
# speedup vs baseline: 1.0620x; 1.0620x over previous
"""Galerkin-attention encoder block on 8 TRN2 NeuronCores — fp8 edition.

Sharding: tokens (N=8192 -> 1024/core). The only cross-core dependency is
the Galerkin contraction scores[b,h] = sum_n k[n] (x) v[n] / N, reduced with
four per-batch 512KB AllReduces that overlap local compute.

Precision plan (validated against a numpy simulation of the reference):
the K/V/Q projections and the first FFN matmul run in fp8 e4m3 with
DoubleRow perf mode (2 fp8 k-subtiles contracted per PE pass = 2x bf16
throughput); the Galerkin contraction, attention apply and second FFN
matmul stay bf16. Weights are pre-scaled by S=1024 (exact power of two)
on the host so w*S lands in e4m3's normal range; the 1/S descale rides
the existing post-matmul activation's `scale` operand. The per-head
LayerNorm after the K/V projections is scale-invariant, so the k/v path
needs no descale at all. Simulated rel err: 1.7e-2 (vs 2.7e-3 bf16).

All device compute runs in "transposed space" (features on partitions,
tokens on the free axis) against host-side pre-transposed x^T (fp8 for
matmul inputs, bf16 for the residual), so the kernel needs no on-device
transposes anywhere.
"""

import numpy as np
import ml_dtypes

B, N, D = 4, 8192, 1024
H, DK = 8, 128
FF = 4096
EPS = 1e-5
N_CORES = 8
NT = N // N_CORES          # tokens per core
KC = D // 128              # feature chunks of 128
FC = FF // 128
SUP = 512                  # tokens per super-tile in phases B1/B2
NSUP = NT // SUP
SUB = 128                  # tokens per sub-tile in phase A
NSUB = SUP // SUB
KC2 = KC // 2              # DoubleRow k-subtile pairs
KF = 4                     # FFN2 k-chunks (of FC=32) computed in fp8
KF2 = KF // 2
FCA = FF // 256 - KF       # bf16 chunks in w2a (first half minus fp8 part)
WS = 1024.0                # fp8 weight prescale (power of two: exact)

_GRAPH_CACHE = {}


def _build(flags, phases=3):
    import concourse.bass as bass
    import concourse.tile as tile
    from concourse import bacc, mybir
    from contextlib import ExitStack

    has_bk, has_bv, has_b2, has_affine = flags
    f32 = mybir.dt.float32
    bf16 = mybir.dt.bfloat16
    fp8 = mybir.dt.float8e4
    DR = mybir.MatmulPerfMode.DoubleRow

    nc = bacc.Bacc("TRN2", target_bir_lowering=False, debug=False,
                   num_devices=N_CORES)

    xTb_d = nc.dram_tensor("xTb", [B, D, NT], bf16, kind="ExternalInput")
    xTq_d = nc.dram_tensor("xTq", [B, D, NT], fp8, kind="ExternalInput")
    delta_d = nc.dram_tensor("delta", [NT], f32, kind="ExternalInput")
    wq_d = nc.dram_tensor("Wq", [D, D], fp8, kind="ExternalInput")
    # Wk/Wv arrive pre-packed from the host in the DoubleRow pair layout
    # [p, kcp, oc, two, col] (see kernel() below).
    wk_d = nc.dram_tensor("Wk", [128, KC2 * 2 * 2 * 512], fp8, kind="ExternalInput")
    wv_d = nc.dram_tensor("Wv", [128, KC2 * 2 * 2 * 512], fp8, kind="ExternalInput")
    w1_d = nc.dram_tensor("W1", [D, FF], fp8, kind="ExternalInput")
    # FFN2 split-K: the first KF2*128 rows of W2 run in fp8 DoubleRow (the
    # rel-err budget allows ~1/8 of the contraction in fp8), the rest bf16.
    # All parts arrive pre-scaled by WS so they share one PSUM accumulation;
    # a single 1/WS descale rides the output path.
    w2f_d = nc.dram_tensor("W2f", [KF2 * 256, D], fp8, kind="ExternalInput")
    w2a_d = nc.dram_tensor("W2a", [FF // 2 - KF2 * 256, D], bf16, kind="ExternalInput")
    w2b_d = nc.dram_tensor("W2b", [FF // 2, D], bf16, kind="ExternalInput")
    bq_d = nc.dram_tensor("bq", [D], f32, kind="ExternalInput")
    b1_d = nc.dram_tensor("b1", [FF], f32, kind="ExternalInput")
    bk_d = nc.dram_tensor("bk", [D], f32, kind="ExternalInput") if has_bk else None
    bv_d = nc.dram_tensor("bv", [D], f32, kind="ExternalInput") if has_bv else None
    b2_d = nc.dram_tensor("b2", [D], f32, kind="ExternalInput") if has_b2 else None
    gamma_d = nc.dram_tensor("gamma", [D], f32, kind="ExternalInput") if has_affine else None
    beta_d = nc.dram_tensor("beta", [D], f32, kind="ExternalInput") if has_affine else None
    out_d = nc.dram_tensor("outT", [B, D, NT], f32, kind="ExternalOutput")

    sub_ = mybir.AluOpType.subtract
    mult = mybir.AluOpType.mult
    ACT = mybir.ActivationFunctionType

    with tile.TileContext(nc) as tc, ExitStack() as ctx:
        singles = ctx.enter_context(tc.tile_pool(name="singles", bufs=1))
        dram = ctx.enter_context(tc.tile_pool(name="dram", bufs=1, space="DRAM"))

        eps_t = singles.tile([128, 1], f32)
        nc.vector.memset(eps_t, EPS)
        delta_sb = singles.tile([128, NT // 128], f32)
        nc.sync.dma_start(out=delta_sb[:], in_=delta_d.ap().rearrange("(g p) -> p g", p=128))
        scores_bf = singles.tile([128, B, H, DK], bf16)

        cc_in = dram.tile([B, 128, H * DK], f32)
        cc_out = [dram.tile([128, H * DK], f32, addr_space="Shared",
                            name=f"cc_out{b}") for b in range(B)]
        h_dram = dram.tile([B, NSUP, 128, FC - KF, SUP], bf16)
        h8_dram = dram.tile([B, NSUP, 128, KF, SUP], fp8)
        x1_dram = dram.tile([B, NSUP, 128, KC, SUP], bf16)

        # W2 halves, reserved below w_ab1 on the pool stack so they
        # survive into B2. w2a's DMA is issued LAST on the scalar queue so
        # the phase-A/B1-critical weights (wq, w1) land first; w2b's is
        # issued at B1 entry on the gpsimd queue.
        w_b2a_cm = tc.tile_pool(name="w_b2a", bufs=1)
        w_b2a = w_b2a_cm.__enter__()
        w2f_sb = w_b2a.tile([128, KF, D], fp8)
        w2a_sb = w_b2a.tile([128, FCA, D], bf16)
        w2b_sb = w_b2a.tile([128, FC // 2, D], bf16)

        # Weights that live through phases A+B1, on the Scalar engine's
        # DMA queue (the Sync queue carries wk/wv/x, which gate the very
        # first matmuls).
        w_ab1_cm = tc.tile_pool(name="w_ab1", bufs=1)
        w_ab1 = w_ab1_cm.__enter__()
        wq_sb = w_ab1.tile([128, KC, D], fp8)
        w1_sb = w_ab1.tile([128, KC, FF], fp8)
        bq_sb = w_ab1.tile([128, KC], f32)
        b1_sb = w_ab1.tile([128, FC], f32)
        nc.scalar.dma_start(out=bq_sb[:], in_=bq_d.ap().rearrange("(m p) -> p m", p=128))
        nc.scalar.dma_start(out=b1_sb[:], in_=b1_d.ap().rearrange("(m p) -> p m", p=128))
        if has_affine:
            gamma_sb = w_ab1.tile([128, D], f32)
            nc.scalar.dma_start(out=gamma_sb[:], in_=gamma_d.ap().to_broadcast([128, D]))
            beta_sb = w_ab1.tile([128, D], f32)
            nc.scalar.dma_start(out=beta_sb[:], in_=beta_d.ap().to_broadcast([128, D]))
        if has_bk:
            bk_sb = w_ab1.tile([128, D], f32)
            nc.scalar.dma_start(out=bk_sb[:], in_=bk_d.ap().to_broadcast([128, D]))
        if has_bv:
            bv_sb = w_ab1.tile([128, D], f32)
            nc.scalar.dma_start(out=bv_sb[:], in_=bv_d.ap().to_broadcast([128, D]))
        def prefetch_b_weights(dep):
            # Run mid-phase-A: at t=0 these ~9MB would contend with the
            # wk/wv/x loads that gate the very first matmuls. The dummy
            # 1-element writes (reading `dep`, a phase-A product) give each
            # prefetch DMA a WAR dependency so the scheduler cannot hoist
            # it to t=0.
            for t_ in (wq_sb, w1_sb, w2f_sb, w2a_sb, w2b_sb):
                nc.vector.tensor_copy(out=t_[:, 0, 0:1], in_=dep[:, 0:1])
            nc.scalar.dma_start(out=wq_sb[:], in_=wq_d.ap().rearrange("(kc p) f -> p kc f", p=128))
            nc.scalar.dma_start(out=w1_sb[:], in_=w1_d.ap().rearrange("(kc p) f -> p kc f", p=128))
            nc.scalar.dma_start(out=w2f_sb[:], in_=w2f_d.ap().rearrange("(kc p) f -> p kc f", p=128))
            nc.scalar.dma_start(out=w2a_sb[:], in_=w2a_d.ap().rearrange("(kc p) f -> p kc f", p=128))
            # NOT on the gpsimd queue: a 4MB SWDGE DMA there delays the
            # AllReduce triggers behind it by ~75us.
            nc.scalar.dma_start(out=w2b_sb[:], in_=w2b_d.ap().rearrange("(kc p) f -> p kc f", p=128))

        xTb_r = [xTb_d.ap()[b].rearrange("(kc p) t -> p kc t", p=128) for b in range(B)]
        xTq_r = [xTq_d.ap()[b].rearrange("(kc p) t -> p kc t", p=128) for b in range(B)]
        xTq_p = [xTq_d.ap()[b].rearrange("(kcp two p) t -> p two kcp t", p=128, two=2)
                 for b in range(B)]

        # ---------------- Phase A: k, v, LN, partial scores, AllReduce ----
        with (
            tc.tile_pool(name="wa", bufs=1) as wa,
            tc.tile_pool(name="a_kvf", bufs=4) as a_kvf,
            tc.tile_pool(name="a_ln", bufs=4) as a_ln,
            tc.tile_pool(name="a_sc", bufs=2) as a_sc,
            tc.tile_pool(name="a_pad", bufs=1) as a_pad,
            tc.tile_pool(name="a_x", bufs=2) as a_x,
            tc.tile_pool(name="a_kvps", bufs=3, space="PSUM") as a_kvps,
            tc.tile_pool(name="a_sps", bufs=1, space="PSUM") as a_sps,
        ):
            # Pair-partner (DoubleRow k-subtile) layout with partner stride
            # 512B: [p, kcp, oc, two, col]. A 1024B partner stride (the
            # plain [p, kc, f] layout) costs ~1.6x on the PE's moving-
            # operand stream.
            wk_sb = wa.tile([128, KC2, 2, 2, 512], fp8)
            nc.sync.dma_start(out=wk_sb[:], in_=wk_d.ap())
            wv_sb = wa.tile([128, KC2, 2, 2, 512], fp8)
            nc.sync.dma_start(out=wv_sb[:], in_=wv_d.ap())
            # Spacer: pushes xtb (stationary operand of the proj matmuls)
            # past the 192KB line so it sits in a different 64KB SBUF
            # subarray than wk/wv (the moving operand). Same-subarray
            # operands serialize LDWEIGHTS with the moving stream
            # (634-760ns vs 437ns per DoubleRow matmul).
            pad = a_pad.tile([128, 32768], mybir.dt.uint8, name="a_pad_t")

            def proj(xtb, tsl, w_sb, bias_sb, tag):
                # psum holds S*(x@W); the LayerNorm right after is
                # scale-invariant so no descale is needed (host passes
                # bk/bv pre-scaled by S).
                ps = a_kvps.tile([128, H, DK], f32, tag="kv")
                for oc in range(2):
                    for kcp in range(KC2):
                        nc.tensor.matmul(
                            ps[:, oc * 4:(oc + 1) * 4, :],
                            lhsT=xtb[:, :, kcp, tsl],
                            rhs=w_sb[:, kcp, oc],
                            start=(kcp == 0), stop=(kcp == KC2 - 1),
                            perf_mode=DR)
                if bias_sb is not None:
                    nc.vector.tensor_add(ps[:, :, :], ps[:, :, :], bias_sb[:])
                return ps

            def layernorm(ps, gsub, with_delta, tag):
                # DVE computes per-head bn_stats; the even/odd halves are
                # combined manually with a handful of [128,H] strided ops
                # (bn_aggr costs ~170ns x 16/subtile in per-op overhead —
                # a third of DVE's phase-A time). The LN apply runs on the
                # Scalar engine as Identity(k*rstd + (-mu*rstd)) so DVE
                # doesn't gate the PE.
                stats = a_ln.tile([128, H, 6], f32, tag="stats", name=f"stats{tag}")
                mv = a_ln.tile([128, H, 2], f32, tag="mv", name=f"mv{tag}")
                for h in range(H):
                    nc.vector.bn_stats(out=stats[:, h, :], in_=ps[:, h, :])
                    nc.vector.bn_aggr(out=mv[:, h, :], in_=stats[:, h, :])
                rstd = a_ln.tile([128, H], f32, tag="rstd", name=f"rstd{tag}")
                nc.scalar.activation(out=rstd[:], in_=mv[:, :, 1], func=ACT.Sqrt,
                                     bias=eps_t[:])
                nc.vector.reciprocal(out=rstd[:], in_=rstd[:])
                out_t = a_kvf.tile([128, D], bf16, tag="kvf", name=f"kvf{tag}")
                if not has_affine:
                    if with_delta:
                        nc.vector.tensor_scalar_mul(
                            out=rstd[:], in0=rstd[:],
                            scalar1=delta_sb[:, gsub:gsub + 1])
                    nmr = a_ln.tile([128, H], f32, tag="nmr", name=f"nmr{tag}")
                    nc.vector.tensor_mul(nmr[:], mv[:, :, 0], rstd[:])
                    nc.vector.tensor_scalar_mul(out=nmr[:], in0=nmr[:], scalar1=-1.0)
                    for h in range(H):
                        nc.scalar.activation(
                            out=out_t[:, h * DK:(h + 1) * DK], in_=ps[:, h, :],
                            func=ACT.Identity,
                            bias=nmr[:, h:h + 1], scale=rstd[:, h:h + 1])
                else:
                    for h in range(H):
                        nc.vector.tensor_scalar(
                            out=out_t[:, h * DK:(h + 1) * DK], in0=ps[:, h, :],
                            scalar1=mv[:, h, 0:1], scalar2=rstd[:, h:h + 1],
                            op0=sub_, op1=mult)
                    nc.vector.tensor_mul(out_t[:], out_t[:], gamma_sb[:])
                    nc.vector.tensor_add(out_t[:], out_t[:], beta_sb[:])
                    if with_delta:
                        nc.vector.tensor_scalar_mul(
                            out=out_t[:], in0=out_t[:],
                            scalar1=delta_sb[:, gsub:gsub + 1])
                return out_t

            for b in range(B):
                scores_ps = a_sps.tile([128, H, DK], f32, tag="scores")
                # Four per-head accumulation groups share each PSUM bank, and
                # a start=True matmul clears its whole bank — zero once and
                # accumulate with start=False instead.
                nc.vector.memset(scores_ps, 0.0)
                prev = None

                def scores_mm(kf, vf, last):
                    for h in range(H):
                        hs = slice(h * DK, (h + 1) * DK)
                        nc.tensor.matmul(
                            scores_ps[:, h, :], lhsT=kf[:, hs], rhs=vf[:, hs],
                            start=False, stop=last,
                            skip_group_check=True)

                for s in range(NSUP):
                    # [p, two, kcp, t]: the DoubleRow pair partner (two) sits
                    # KC2*SUP = 2KB away. A 512B partner stride on the
                    # STATIONARY operand drops the PE to one fp8 row/cycle
                    # (634ns vs 437ns per matmul); >=1KB runs double-pumped.
                    xtb = a_x.tile([128, 2, KC2, SUP], fp8, tag="xtb")
                    for two in range(2):
                        nc.sync.dma_start(
                            out=xtb[:, two],
                            in_=xTq_p[b][:, two, :, s * SUP:(s + 1) * SUP])
                    if b == 0 and s == 1:
                        prefetch_b_weights(prev[0])
                    for sb in range(NSUB):
                        gsub = s * NSUB + sb
                        tsl = bass.ts(sb, SUB)

                        k_ps = proj(xtb, tsl, wk_sb, bk_sb if has_bk else None, "k")
                        v_ps = proj(xtb, tsl, wv_sb, bv_sb if has_bv else None, "v")
                        # Software-pipelined: the scores contraction for the
                        # PREVIOUS sub-tile issues here, so the PE never
                        # waits on the current sub-tile's LN chain.
                        if prev is not None:
                            scores_mm(*prev, last=False)
                        kf = layernorm(k_ps, gsub, True, "k")
                        vf = layernorm(v_ps, gsub, False, "v")
                        prev = (kf, vf)

                scores_mm(*prev, last=True)
                sc_sb = a_sc.tile([128, H * DK], f32, tag="scsb")
                nc.vector.tensor_copy(out=sc_sb[:], in_=scores_ps[:, :, :])
                nc.sync.dma_start(out=cc_in[b], in_=sc_sb[:])
                nc.gpsimd.collective_compute(
                    "AllReduce", mybir.AluOpType.add,
                    replica_groups=[list(range(N_CORES))],
                    ins=[cc_in[b].opt()], outs=[cc_out[b].opt()])

        if phases < 1:
            # Debug: dump reduced scores straight to outT and stop.
            with tc.tile_pool(name="dbg", bufs=1) as dbg:
                sc_dbg = dbg.tile([128, B, H * DK], f32)
                for b in range(B):
                    nc.sync.dma_start(out=sc_dbg[:, b, :], in_=cc_out[b])
                    nc.sync.dma_start(out=out_d.ap()[b][0:128, :],
                                      in_=sc_dbg[:, b, 0:NT])
            w_ab1_cm.__exit__(None, None, None)
            w_b2a_cm.__exit__(None, None, None)
            nc.finalize()
            return nc

        # ---------------- Phase B1: qT, attn, x1T, first FFN matmul -------
        with (
            tc.tile_pool(name="b1_x", bufs=2) as b1_x,
            tc.tile_pool(name="b1_q", bufs=1) as b1_q,
            tc.tile_pool(name="b1_x1", bufs=2) as b1_x1,
            tc.tile_pool(name="b1_x1q", bufs=2) as b1_x1q,
            tc.tile_pool(name="b1_h", bufs=2) as b1_h,
            tc.tile_pool(name="b1_qps", bufs=2, space="PSUM") as b1_qps,
            tc.tile_pool(name="b1_aps", bufs=2, space="PSUM") as b1_aps,
            tc.tile_pool(name="b1_hps", bufs=3, space="PSUM") as b1_hps,
        ):
            b1_sc_cm = tc.tile_pool(name="b1_sc", bufs=2)
            b1_sc = b1_sc_cm.__enter__()
            for b in range(B):
                # Lazy per-batch scores copy: batch b's copy only waits on
                # batch b's AllReduce. On GPSIMD (not DVE): the in-order
                # DVE queue must not head-of-line block phase-A/B1 vector
                # work on an AllReduce that hasn't completed yet.
                sc_f = b1_sc.tile([128, H * DK], f32, tag="scf")
                nc.sync.dma_start(out=sc_f[:], in_=cc_out[b])
                nc.gpsimd.tensor_copy(out=scores_bf[:, b, :, :], in_=sc_f[:])
                for s in range(NSUP):
                    xtq = b1_x.tile([128, KC, SUP], fp8, tag="xtq")
                    nc.sync.dma_start(out=xtq[:], in_=xTq_r[b][:, :, s * SUP:(s + 1) * SUP])
                    xtb = b1_x.tile([128, KC, SUP], bf16, tag="xtb1")
                    nc.sync.dma_start(out=xtb[:], in_=xTb_r[b][:, :, s * SUP:(s + 1) * SUP])

                    qt = b1_q.tile([128, H, SUP], bf16, tag="qt")
                    for m in range(KC):
                        q_ps = b1_qps.tile([128, SUP], f32, tag="qps")
                        for kp in range(0, KC, 2):
                            nc.tensor.matmul(
                                q_ps[:], lhsT=wq_sb[:, kp:kp + 2, m * 128:(m + 1) * 128],
                                rhs=xtq[:, kp:kp + 2, :],
                                start=(kp == 0), stop=(kp == KC - 2),
                                perf_mode=DR)
                        nc.scalar.activation(out=qt[:, m, :], in_=q_ps[:],
                                             func=ACT.Identity,
                                             bias=bq_sb[:, m:m + 1], scale=1.0 / WS)

                    x1 = b1_x1.tile([128, KC, SUP], bf16, tag="x1")
                    x1q = b1_x1q.tile([128, KC, SUP], fp8, tag="x1q")
                    for h in range(H):
                        a_ps = b1_aps.tile([128, SUP], f32, tag="aps")
                        nc.tensor.matmul(a_ps[:], lhsT=scores_bf[:, b, h, :],
                                         rhs=qt[:, h, :], start=True, stop=True)
                        nc.vector.tensor_add(x1[:, h, :], a_ps[:], xtb[:, h, :])
                        # Second add straight to fp8 so FFN1's first matmul
                        # only waits on chunks 0-1, not a full-tile copy.
                        nc.vector.tensor_add(x1q[:, h, :], a_ps[:], xtb[:, h, :])
                    nc.gpsimd.dma_start(out=x1_dram[b, s], in_=x1[:])

                    hf8 = b1_h.tile([128, KF, SUP], fp8, tag="hf8")
                    for g in range(4):
                        hsb = b1_h.tile([128, 8, SUP], bf16, tag="hsb")
                        for mm in range(8):
                            m = g * 8 + mm
                            h_ps = b1_hps.tile([128, SUP], f32, tag="hps")
                            for kp in range(0, KC, 2):
                                nc.tensor.matmul(
                                    h_ps[:], lhsT=w1_sb[:, kp:kp + 2, m * 128:(m + 1) * 128],
                                    rhs=x1q[:, kp:kp + 2, :],
                                    start=(kp == 0), stop=(kp == KC - 2),
                                    perf_mode=DR)
                            # First KF chunks feed FFN2's fp8 part; the rest
                            # stay bf16. (hsb[:, mm] for m < KF is dead.)
                            if m < KF:
                                nc.scalar.activation(out=hf8[:, m, :], in_=h_ps[:],
                                                     func=ACT.Silu,
                                                     bias=b1_sb[:, m:m + 1], scale=1.0 / WS)
                            else:
                                nc.scalar.activation(out=hsb[:, mm, :], in_=h_ps[:],
                                                     func=ACT.Silu,
                                                     bias=b1_sb[:, m:m + 1], scale=1.0 / WS)
                        if g == 0:
                            nc.gpsimd.dma_start(out=h8_dram[b, s], in_=hf8[:])
                            nc.gpsimd.dma_start(out=h_dram[b, s, :, 0:8 - KF, :],
                                                in_=hsb[:, KF:8, :])
                        else:
                            nc.gpsimd.dma_start(
                                out=h_dram[b, s, :, g * 8 - KF:(g + 1) * 8 - KF, :],
                                in_=hsb[:])

            b1_sc_cm.__exit__(None, None, None)

        w_ab1_cm.__exit__(None, None, None)

        if phases < 2:
            w_b2a_cm.__exit__(None, None, None)
            # Debug: dump x1 straight to outT and stop.
            with tc.tile_pool(name="dbg2", bufs=2) as dbg2:
                for b in range(B):
                    for s in range(NSUP):
                        x1d = dbg2.tile([128, KC, SUP], bf16, tag="x1d")
                        nc.sync.dma_start(out=x1d[:], in_=x1_dram[b, s])
                        x1f = dbg2.tile([128, KC, SUP], f32, tag="x1f")
                        nc.vector.tensor_copy(out=x1f[:], in_=x1d[:])
                        out_r_b = out_d.ap()[b].rearrange("(m p) t -> p m t", p=128)
                        nc.sync.dma_start(out=out_r_b[:, :, s * SUP:(s + 1) * SUP],
                                          in_=x1f[:])
            nc.finalize()
            return nc

        # ---------------- Phase B2: yT + residual -------------------------
        with (
            tc.tile_pool(name="w_b2", bufs=1) as w_b2,
            tc.tile_pool(name="b2_h", bufs=2) as b2_h,
            tc.tile_pool(name="b2_x1", bufs=2) as b2_x1,
            tc.tile_pool(name="b2_o", bufs=2) as b2_o,
            tc.tile_pool(name="b2_yps", bufs=4, space="PSUM") as b2_yps,
        ):
            if has_b2:
                b2_sb = w_b2.tile([128, KC], f32)
                nc.sync.dma_start(out=b2_sb[:], in_=b2_d.ap().rearrange("(m p) -> p m", p=128))

            out_r = [out_d.ap()[b].rearrange("(m p) t -> p m t", p=128) for b in range(B)]
            FCH = FC // 2
            for b in range(B):
                for s in range(NSUP):
                    hf = b2_h.tile([128, KF, SUP], fp8, tag="hf")
                    nc.sync.dma_start(out=hf[:], in_=h8_dram[b, s])
                    ht = b2_h.tile([128, FC - KF, SUP], bf16, tag="ht")
                    # Chunked readback: the first supertile's matmuls start
                    # as soon as the first 7 chunks land instead of waiting
                    # for the full 3.5MB transfer.
                    for hc in range(0, FC - KF, 7):
                        nc.sync.dma_start(out=ht[:, hc:hc + 7],
                                          in_=h_dram[b, s, :, hc:hc + 7, :])
                    x1r = b2_x1.tile([128, KC, SUP], bf16, tag="x1r")
                    nc.sync.dma_start(out=x1r[:], in_=x1_dram[b, s])
                    ot = b2_o.tile([128, KC, SUP], f32, tag="ot")
                    for m in range(KC):
                        y_ps = b2_yps.tile([128, SUP], f32, tag="yps")
                        for kp in range(KF2):
                            nc.tensor.matmul(
                                y_ps[:],
                                lhsT=w2f_sb[:, 2 * kp:2 * kp + 2, m * 128:(m + 1) * 128],
                                rhs=hf[:, 2 * kp:2 * kp + 2, :],
                                start=(kp == 0), stop=False, perf_mode=DR)
                        for kc in range(FCA):
                            nc.tensor.matmul(
                                y_ps[:], lhsT=w2a_sb[:, kc, m * 128:(m + 1) * 128],
                                rhs=ht[:, kc, :],
                                start=False, stop=False)
                        for kc in range(FCH):
                            nc.tensor.matmul(
                                y_ps[:], lhsT=w2b_sb[:, kc, m * 128:(m + 1) * 128],
                                rhs=ht[:, FCA + kc, :],
                                start=False, stop=(kc == FCH - 1))
                        # W2 parts are all pre-scaled by WS; descale on the
                        # (otherwise idle) Scalar engine, then add the
                        # residual on DVE.
                        ys = b2_o.tile([128, SUP], f32, tag="ys")
                        nc.scalar.activation(
                            out=ys[:], in_=y_ps[:], func=ACT.Identity,
                            bias=b2_sb[:, m:m + 1] if has_b2 else 0.0,
                            scale=1.0 / WS)
                        nc.vector.tensor_add(ot[:, m, :], ys[:], x1r[:, m, :])
                        # Per-chunk writeout so the kernel's drain tail is
                        # one 256KB DMA, not a 2MB one.
                        nc.gpsimd.dma_start(
                            out=out_r[b][:, m, s * SUP:(s + 1) * SUP],
                            in_=ot[:, m, :])

        w_b2a_cm.__exit__(None, None, None)

    nc.finalize()
    return nc


def _get_graph(flags):
    if flags not in _GRAPH_CACHE:
        _GRAPH_CACHE[flags] = _build(flags)
    return _GRAPH_CACHE[flags]


def kernel(x, delta_x, Wq, bq, Wk, bk, Wv, bv, gamma_k, beta_k, W1, b1, W2, b2,
           _trace=False):
    from concourse.bass_utils import run_bass_kernel_spmd

    bf = ml_dtypes.bfloat16
    f8 = ml_dtypes.float8_e4m3
    x = np.asarray(x, np.float32)
    delta_x = np.asarray(delta_x, np.float32)
    Wq, Wk, Wv = (np.asarray(w, np.float32) for w in (Wq, Wk, Wv))
    W1, W2 = np.asarray(W1, np.float32), np.asarray(W2, np.float32)
    bq, bk, bv = (np.asarray(v, np.float32) for v in (bq, bk, bv))
    b1, b2 = np.asarray(b1, np.float32), np.asarray(b2, np.float32)
    gamma_k = np.asarray(gamma_k, np.float32)
    beta_k = np.asarray(beta_k, np.float32)

    has_bk = bool(np.any(bk))
    has_bv = bool(np.any(bv))
    has_b2 = bool(np.any(b2))
    has_affine = not (np.all(gamma_k == 1.0) and np.all(beta_k == 0.0))
    flags = (has_bk, has_bv, has_b2, has_affine)
    nc = _get_graph(flags)

    def q8(w):
        return np.clip(w * np.float32(WS), -240, 240).astype(f8)

    def pack_pairs(w8):
        # [D, D] -> [p, kcp, oc, two, col] flattened to [128, 8192]: the
        # DoubleRow pair partner (two) sits 512B from its mate so the PE's
        # moving-operand stream stays on fast SBUF strides.
        w5 = w8.reshape(KC2, 2, 128, 2, 512).transpose(2, 0, 3, 1, 4)
        return np.ascontiguousarray(w5).reshape(128, KC2 * 2 * 2 * 512)

    wq_8, w1_8 = q8(Wq), q8(W1)
    wk_8, wv_8 = pack_pairs(q8(Wk)), pack_pairs(q8(Wv))
    w2f_8 = q8(W2[0:KF2 * 256])
    w2a_b = (W2[KF2 * 256:FF // 2] * np.float32(WS)).astype(bf)
    w2b_b = (W2[FF // 2:] * np.float32(WS)).astype(bf)
    delta_pre = (delta_x / np.float32(N)).astype(np.float32)

    in_maps = []
    for c in range(N_CORES):
        t0 = c * NT
        xT = np.ascontiguousarray(x[:, t0:t0 + NT, :].transpose(0, 2, 1))
        m = {"xTb": xT.astype(bf),
             "xTq": np.clip(xT, -240, 240).astype(f8),
             "delta": np.ascontiguousarray(delta_pre[t0:t0 + NT]),
             "Wq": wq_8, "Wk": wk_8, "Wv": wv_8, "W1": w1_8,
             "W2f": w2f_8, "W2a": w2a_b, "W2b": w2b_b,
             "bq": bq, "b1": b1}
        if has_bk:
            m["bk"] = bk * np.float32(WS)
        if has_bv:
            m["bv"] = bv * np.float32(WS)
        if has_b2:
            m["b2"] = b2
        if has_affine:
            m["gamma"] = gamma_k.reshape(D).copy()
            m["beta"] = beta_k.reshape(D).copy()
        in_maps.append(m)

    res = run_bass_kernel_spmd(nc, in_maps, core_ids=list(range(N_CORES)),
                               trace=_trace)

    out = np.empty((B, N, D), np.float32)
    for c in range(N_CORES):
        t0 = c * NT
        out[:, t0:t0 + NT, :] = res.results[c]["outT"].transpose(0, 2, 1)
    if _trace:
        return out, res
    return out
